# revision 11
# baseline (speedup 1.0000x reference)
"""Adaptive attention kernel for Trainium2, SPMD over 8 NeuronCores.

Problem: out = softmax(alpha*scores + (1-alpha)*row_mean(scores)) @ V with
scores = Q K^T, per (batch, head).  Since row_mean is constant along the
softmax axis, softmax(alpha*s + c_row) == softmax(alpha*s): the blend drops
out and the kernel computes softmax(alpha * Q K^T) V exactly.

Sharding: B*H = 32 head-slices, 4 per core, fully data-parallel (no
collectives).  Per head the kernel computes S^T = (alpha*Q K^T)^T tiles on
the TensorEngine, exponentiates on the ScalarEngine (constant safety bias,
softmax-shift-invariant), accumulates the softmax denominator with an
all-ones matmul and the O^T = V^T P^T product on the TensorEngine, then
normalizes and transposes O^T back on-chip.
"""

import os
from contextlib import ExitStack

import numpy as np

import concourse.bass as bass
import concourse.mybir as mybir
import concourse.tile as tile
from concourse import bacc
from concourse import bass_utils
from concourse.masks import make_identity

B, H, S, D = 2, 16, 2048, 128
N_CORES = 8
HEADS_PER_CORE = (B * H) // N_CORES  # 4

FP32 = mybir.dt.float32
BF16 = mybir.dt.bfloat16

# exp safety bias: exp(alpha*s - BIAS_C*alpha).  alpha*s ~ N(0, (alpha*sqrt(D))^2),
# |alpha*s| < ~70*alpha over 1e8 samples, so exponents stay within fp32/bf16 range
# for any alpha in [0, 1].  Softmax is invariant to the constant shift.
BIAS_C = 35.0

SQ_GROUP = 512            # moving free dim per matmul (one PSUM bank of fp32)
N_GROUPS = S // SQ_GROUP  # 4
N_CHUNKS = S // 128       # 16 key chunks

# Chunks whose exp runs on the DVE via the 16-bit Schraudolph bit-trick
# (i16 = arg*2^7/ln2 + B, bitcast bf16 ~= exp(arg), ~2% rms error) instead
# of the ACT engine -- balances the exp load across both engines.  Chosen
# away from c0/c1 (gp warmup) and c12..c15 (Z tail dependencies).
SCH_CHUNKS = frozenset(
    int(c) for c in os.environ.get("KERNEL_SCH_CHUNKS", "3,6,9").split(",") if c != ""
)
A16_SCH = 2.0 ** 7 / 0.6931471805599453        # 2^7 / ln2
B16_SCH = 127.0 * 2 ** 7 - 366393.0 / 65536.0  # bf16 exp bias - C_schraudolph

# matmul input dtype config
# "f32"  : plain fp32 (4 cycles/row, exact)
# "f32r" : fp32 replicated/round mode (1 cycle/row at N>=256, reduced precision)
QK_DTYPE = os.environ.get("KERNEL_QK_DTYPE", "bf16")
P_DTYPE = os.environ.get("KERNEL_P_DTYPE", "bf16")  # "f32" or "bf16"


def _qk_dt():
    if QK_DTYPE == "f32r":
        return mybir.dt.float32r
    if QK_DTYPE == "bf16":
        return BF16
    return FP32


def build_core_graph():
    """Build the per-core Bass graph (4 heads, full attention per head)."""
    nc = bacc.Bacc(
        "TRN2", target_bir_lowering=False, debug=False, enable_asserts=False
    )
    p_dt = BF16 if P_DTYPE == "bf16" else FP32

    q_d = nc.declare_dram_parameter("q", [HEADS_PER_CORE, S, D], FP32, isOutput=False)
    k_d = nc.declare_dram_parameter("k", [HEADS_PER_CORE, S, D], FP32, isOutput=False)
    v_d = nc.declare_dram_parameter("v", [HEADS_PER_CORE, S, D], FP32, isOutput=False)
    a_d = nc.declare_dram_parameter("alpha", [1, 1], FP32, isOutput=False)
    o_d = nc.declare_dram_parameter("out", [HEADS_PER_CORE, S, D], FP32, isOutput=True)

    q_ap, k_ap, v_ap, o_ap = q_d.ap(), k_d.ap(), v_d.ap(), o_d.ap()

    with tile.TileContext(nc) as tc, ExitStack() as ctx:
        const = ctx.enter_context(tc.tile_pool(name="const", bufs=1))
        nat = ctx.enter_context(tc.tile_pool(name="nat", bufs=4))
        qkt = ctx.enter_context(tc.tile_pool(name="qkt", bufs=2))
        vpool = ctx.enter_context(tc.tile_pool(name="vpool", bufs=2))
        ppool = ctx.enter_context(tc.tile_pool(name="ppool", bufs=8))
        epil = ctx.enter_context(tc.tile_pool(name="epil", bufs=3))
        outp = ctx.enter_context(tc.tile_pool(name="outp", bufs=4))
        # PSUM budget (8 banks): sp [128,1024] x2 bufs = 4, z [128,1024] = 2,
        # o [128,1024] = 2; the 1-bank "s" transpose tiles ride the same pool
        ps_s = ctx.enter_context(tc.tile_pool(name="ps_s", bufs=2, space="PSUM"))
        ps_z = ctx.enter_context(tc.tile_pool(name="ps_z", bufs=1, space="PSUM"))
        ps_o = ctx.enter_context(tc.tile_pool(name="ps_o", bufs=1, space="PSUM"))
        dpool = ctx.enter_context(tc.tile_pool(name="dram", bufs=4, space="DRAM"))
        zqp = ctx.enter_context(tc.tile_pool(name="zqp", bufs=8))

        GP = 1024  # paired sq-group width (2 PSUM banks)
        N_GP = S // GP

        # PE "filler" tasks: one-bank transpose batches (Q/K input transposes
        # for the NEXT head, output transposes for the PREVIOUS sq-group)
        # injected between compute matmuls so the PE never idles long enough
        # to stall the pipeline or re-throttle HAM.
        fillers = []

        def _head_permuted(h):
            # heads loaded through the PE-transpose batch path use the
            # contiguous (permuted) layout; xbar-path heads are unpermuted
            return QK_DTYPE != "bf16" or h == 0

        def issue_head_loads(h):
            """Issue all DMA loads for head h and return (qT, kT, v_sb) plus
            filler tasks that transpose the loaded chunks into qT/kT."""
            qT = [qkt.tile([128, GP], _qk_dt(), tag=f"qT{i}", name=f"qT{i}") for i in range(2)]
            kT = [qkt.tile([128, GP], _qk_dt(), tag=f"kT{i}", name=f"kT{i}") for i in range(2)]
            v_sb = vpool.tile([128, N_CHUNKS, 128], p_dt, tag="v")
            perm = _head_permuted(h)

            def load_v():
                if perm:
                    v_src = v_ap[h].rearrange("(u p t) d -> p u t d", u=2, t=8)
                    v_dst = v_sb.rearrange("p (u t) d -> p u t d", u=2)
                else:
                    v_src = v_ap[h].rearrange("(c p) d -> p c d", p=128)
                    v_dst = v_sb
                if p_dt == FP32:
                    nc.sync.dma_start(out=v_dst, in_=v_src)
                else:
                    nc.gpsimd.dma_start(out=v_dst, in_=v_src)  # SWDGE casts
            tasks = []
            if QK_DTYPE == "bf16" and h > 0:
                # steady-state path: SWDGE cast f32->bf16 into DRAM scratch,
                # then whole-half 2-byte DMA xbar transposes — zero PE/DVE
                # work, fully hidden behind the previous head's compute
                qsc = dpool.tile([S, 128], BF16, tag="sc")
                ksc = dpool.tile([S, 128], BF16, tag="sc")
                nc.gpsimd.dma_start(out=qsc[:GP], in_=q_ap[h, :GP])
                nc.gpsimd.dma_start(out=ksc[:GP], in_=k_ap[h, :GP])
                nc.sync.dma_start_transpose(out=qT[0], in_=qsc[:GP])
                nc.sync.dma_start_transpose(out=kT[0], in_=ksc[:GP])
                nc.gpsimd.dma_start(out=ksc[GP:], in_=k_ap[h, GP:])
                nc.gpsimd.dma_start(out=qsc[GP:], in_=q_ap[h, GP:])
                nc.sync.dma_start_transpose(out=kT[1], in_=ksc[GP:])
                nc.sync.dma_start_transpose(out=qT[1], in_=qsc[GP:])
                load_v()
                return qT, kT, v_sb, tasks
            batches = [("q", 0), ("k", 0), ("k", 1), ("q", 1)]
            for name, hf in batches:
                src_ap, dsts = (q_ap, qT) if name == "q" else (k_ap, kT)
                xn = nat.tile([128, 8, 128], FP32, tag="qknat")
                eng = nc.sync if name == "q" else nc.scalar
                # contiguous 4KB-per-partition read; the implied permutation
                # (row p*8+t -> column t*128+p) is undone in the V layout and
                # the output store AP below
                eng.dma_start(
                    out=xn,
                    in_=src_ap[h, hf * GP:(hf + 1) * GP, :].rearrange(
                        "(p t) d -> p t d", p=128
                    ),
                )

                def mk(xn=xn, dsts=dsts, hf=hf):
                    tp = ps_s.tile([128, GP], FP32, tag="sp")
                    for j in range(8):
                        nc.tensor.transpose(
                            tp[:, j * 128:(j + 1) * 128], xn[:, j, :], ident
                        )
                    nc.vector.tensor_copy(out=dsts[hf], in_=tp)

                tasks.append(mk)
            load_v()
            return qT, kT, v_sb, tasks

        def emit_epilogue(h, gp, zp, op):
            """Normalize O^T/Z and write out.  bf16 path: normalize to bf16,
            transpose via DMA xbar, SWDGE cast-store — no PE/DVE epilogue
            work beyond recip+mul (halved so the O bank frees early)."""
            rz = epil.tile([128, GP], FP32, tag="rz")
            on_dt = BF16 if QK_DTYPE == "bf16" else FP32
            on = epil.tile([128, GP], on_dt, tag="on")
            for i in range(2):
                hs = slice(i * 512, (i + 1) * 512)
                nc.vector.reciprocal_approx_fast(out=rz[:, hs], in_=zp[:, hs])
                nc.vector.tensor_mul(on[:, hs], op[:, hs], rz[:, hs])
            tasks = []
            bf = QK_DTYPE == "bf16"
            o_dt = BF16 if bf else FP32
            idw = ident_bf if bf else ident
            for half4 in range(2):

                def mk(h=h, gp=gp, on=on, half4=half4):
                    to = ps_s.tile([128, 512], o_dt, tag="sp")
                    for t in range(4):
                        c0 = half4 * 512 + t * 128
                        nc.tensor.transpose(
                            to[:, t * 128:(t + 1) * 128], on[:, c0:c0 + 128], idw
                        )
                    osb = outp.tile([128, 4, 128], o_dt, tag="osb")
                    nc.vector.tensor_copy(out=osb, in_=to)
                    if _head_permuted(h):
                        o_dst = o_ap[h, gp * GP:(gp + 1) * GP, :].rearrange(
                            "(p u t) d -> p u t d", p=128, u=2
                        )[:, half4, :, :]
                    else:
                        r0 = gp * GP + half4 * 512
                        o_dst = o_ap[h, r0:r0 + 512, :].rearrange(
                            "(t p) d -> p t d", p=128
                        )
                    if bf:
                        nc.gpsimd.dma_start(out=o_dst, in_=osb)
                    else:
                        nc.sync.dma_start(out=o_dst, in_=osb)

                tasks.append(mk)
            return tasks

        # ---- constants first: the tiny alpha DMA must not queue behind the
        # bulk head-0 loads (it gates bias -> exp table -> the whole pipeline)
        alpha_bc = const.tile([128, 1], FP32)
        nc.sync.dma_start(out=alpha_bc, in_=a_d.ap().to_broadcast([128, 1]))
        bias_sb = const.tile([128, 1], FP32)
        nc.vector.tensor_scalar_mul(bias_sb, alpha_bc, -BIAS_C)
        # Schraudolph affine constants with alpha folded in:
        # i16 = s*(alpha*A16) + (B16 - BIAS_C*alpha*A16)
        sch_scale = const.tile([128, 1], FP32)
        nc.vector.tensor_scalar_mul(sch_scale, alpha_bc, A16_SCH)
        sch_bias = const.tile([128, 1], FP32)
        nc.vector.tensor_scalar(
            sch_bias, alpha_bc, -BIAS_C * A16_SCH, B16_SCH,
            mybir.AluOpType.mult, mybir.AluOpType.add,
        )
        ident = const.tile([128, 128], FP32)
        make_identity(nc, ident)
        ident_bf = const.tile([128, 128], BF16)
        nc.vector.tensor_copy(out=ident_bf, in_=ident)
        ones_w = const.tile([128, 128], p_dt)
        nc.vector.memset(ones_w, 1.0)
        ones_bf = const.tile([128, 128], BF16)
        nc.vector.memset(ones_bf, 1.0)
        ones_r = const.tile([128, 128], mybir.dt.float32r)
        nc.vector.tensor_copy(out=ones_r, in_=ones_bf)

        # ---- head 0 loads
        qT, kT, v_sb, tasks = issue_head_loads(0)

        for t in tasks[:2]:
            t()
        fillers.extend(tasks[2:])

        for h in range(HEADS_PER_CORE):
            nxt = None
            for gp in range(N_GP):
                if h + 1 < HEADS_PER_CORE and gp == N_GP - 1:
                    # prefetch next head: DMAs now, transposes become fillers
                    nxt = issue_head_loads(h + 1)
                    fillers.extend(nxt[3])
                zp = ps_z.tile([128, GP], FP32, tag="z")
                op = ps_o.tile([128, GP], FP32, tag="o")

                def consume(c, pt):
                    # AV accumulation for chunk c (after exp(c))
                    for i in range(2):
                        nc.tensor.matmul(
                            op[:, i * 512:(i + 1) * 512], lhsT=v_sb[:, c, :],
                            rhs=pt[:, i * 512:(i + 1) * 512],
                            start=(c == 0), stop=(c == N_CHUNKS - 1),
                        )

                def z_mm(zq, start, stop):
                    # Z accumulation: ones-matmul over a DVE-reduced tile
                    for i in range(2):
                        nc.tensor.matmul(
                            zp[:, i * 512:(i + 1) * 512], lhsT=ones_bf,
                            rhs=zq[:, i * 512:(i + 1) * 512],
                            start=start, stop=stop,
                        )

                # software pipeline: PE issues S(c) while ACT exps c-1 and PE
                # consumes (AV) c-1.  Z denominator: DVE reduction tree over
                # exp tiles — chunks c0..c11 tree-reduce to one tile (3 ones-
                # matmul-equivalents saved vs pair mode), c12..c15 stay as two
                # pair tiles so the PE never waits long on a late DVE add.
                pend = []
                prev_pt = None
                l1 = []   # pair sums awaiting L2 combine (c0..c11 only)
                l2 = []   # quad sums awaiting L3 combine
                tree_a = [None]  # final c0..c11 reduction
                tail = []  # pair tiles fed directly to z_mm (c12/13, c14/15)
                for c in range(N_CHUNKS):
                    sp = ps_s.tile([128, GP], FP32, tag="sp")
                    kw = kT[c // 8][:, (c % 8) * 128:(c % 8) * 128 + 128]
                    for i in range(2):
                        nc.tensor.matmul(
                            sp[:, i * 512:(i + 1) * 512], lhsT=kw,
                            rhs=qT[gp][:, i * 512:(i + 1) * 512],
                            start=True, stop=True,
                        )
                    if c >= 2 and fillers:
                        fillers.pop(0)()
                    if c == 14:
                        # fold the leftover quad (c8..c11) into the tree, then
                        # start Z; tree_a is ready well before this point
                        for rem in l2:
                            t2 = zqp.tile([128, GP], BF16, tag="zq")
                            nc.vector.tensor_add(t2, tree_a[0], rem)
                            tree_a[0] = t2
                        l2.clear()
                        z_mm(tree_a[0], start=True, stop=False)
                    if len(pend) >= 2:
                        consume(*pend.pop(0))
                    pt = ppool.tile([128, GP], p_dt, tag="p")
                    if c in SCH_CHUNKS:
                        # exp on DVE: fused affine -> round-to-nearest i16,
                        # bitcast bf16 == Schraudolph exp of alpha*s - bias
                        nc.vector.tensor_scalar(
                            pt.bitcast(mybir.dt.int16), sp, sch_scale, sch_bias,
                            mybir.AluOpType.mult, mybir.AluOpType.add,
                        )
                    else:
                        nc.scalar.activation(
                            pt, sp, mybir.ActivationFunctionType.Exp,
                            bias=bias_sb, scale=alpha_bc,
                        )
                    if c % 2 == 1:
                        zq = zqp.tile([128, GP], BF16, tag="zq")
                        nc.vector.tensor_add(zq, prev_pt, pt)
                        if c < 12:
                            l1.append(zq)
                            if len(l1) == 2:
                                q = zqp.tile([128, GP], BF16, tag="zq")
                                nc.vector.tensor_add(q, l1[0], l1[1])
                                l1.clear()
                                l2.append(q)
                                if len(l2) == 2:
                                    t = zqp.tile([128, GP], BF16, tag="zq")
                                    nc.vector.tensor_add(t, l2[0], l2[1])
                                    l2.clear()
                                    if tree_a[0] is None:
                                        tree_a[0] = t
                                    else:
                                        t2 = zqp.tile([128, GP], BF16, tag="zq")
                                        nc.vector.tensor_add(t2, tree_a[0], t)
                                        tree_a[0] = t2
                        else:
                            tail.append(zq)
                    prev_pt = pt
                    pend.append((c, pt))
                z_mm(tail[0], start=False, stop=False)
                for pc in pend:
                    consume(*pc)
                z_mm(tail[1], start=False, stop=True)
                fillers.extend(emit_epilogue(h, gp, zp, op))
            if nxt is not None:
                qT, kT, v_sb = nxt[0], nxt[1], nxt[2]

        # drain remaining fillers (last head's output transposes)
        for t in fillers:
            t()

    nc.compile()
    return nc


_NC_CACHE = None
LAST_RESULT = {"exec_time_ns": None}


def _get_nc():
    global _NC_CACHE
    if _NC_CACHE is None:
        _NC_CACHE = build_core_graph()
    return _NC_CACHE


def kernel(q, k, v, alpha):
    q = np.ascontiguousarray(np.asarray(q, dtype=np.float32)).reshape(B * H, S, D)
    k = np.ascontiguousarray(np.asarray(k, dtype=np.float32)).reshape(B * H, S, D)
    v = np.ascontiguousarray(np.asarray(v, dtype=np.float32)).reshape(B * H, S, D)
    a = np.asarray(alpha, dtype=np.float32).reshape(1, 1)

    nc = _get_nc()
    in_maps = []
    for i in range(N_CORES):
        sl = slice(i * HEADS_PER_CORE, (i + 1) * HEADS_PER_CORE)
        in_maps.append({
            "q": np.ascontiguousarray(q[sl]),
            "k": np.ascontiguousarray(k[sl]),
            "v": np.ascontiguousarray(v[sl]),
            "alpha": a,
        })

    trace = os.environ.get("KERNEL_TRACE", "0") == "1"
    res = bass_utils.run_bass_kernel_spmd(
        nc, in_maps, core_ids=list(range(N_CORES)), trace=trace
    )
    LAST_RESULT["exec_time_ns"] = res.exec_time_ns
    LAST_RESULT["res"] = res

    out = np.stack([res.results[i]["out"] for i in range(N_CORES)])
    return out.reshape(B, H, S, D).astype(np.float32)



# revision 18
# speedup vs baseline: 1.2500x; 1.2500x over previous
"""Adaptive attention kernel for Trainium2, SPMD over 8 NeuronCores.

Problem: out = softmax(alpha*scores + (1-alpha)*row_mean(scores)) @ V with
scores = Q K^T, per (batch, head).  Since row_mean is constant along the
softmax axis, softmax(alpha*s + c_row) == softmax(alpha*s): the blend drops
out and the kernel computes softmax(alpha * Q K^T) V exactly.

Sharding: B*H = 32 head-slices, 4 per core, fully data-parallel (no
collectives).  Per head the kernel computes S^T = (alpha*Q K^T)^T tiles on
the TensorEngine, exponentiates on the ScalarEngine (constant safety bias,
softmax-shift-invariant), accumulates the softmax denominator with an
all-ones matmul and the O^T = V^T P^T product on the TensorEngine, then
normalizes and transposes O^T back on-chip.
"""

import os
from contextlib import ExitStack

import numpy as np

import concourse.bass as bass
import concourse.mybir as mybir
import concourse.tile as tile
from concourse import bacc
from concourse import bass_utils
from concourse.masks import make_identity

B, H, S, D = 2, 16, 2048, 128
N_CORES = 8
HEADS_PER_CORE = (B * H) // N_CORES  # 4

FP32 = mybir.dt.float32
BF16 = mybir.dt.bfloat16

# exp safety bias: exp(alpha*s - BIAS_C*alpha).  alpha*s ~ N(0, (alpha*sqrt(D))^2),
# |alpha*s| < ~70*alpha over 1e8 samples, so exponents stay within fp32/bf16 range
# for any alpha in [0, 1].  Softmax is invariant to the constant shift.
BIAS_C = 35.0

SQ_GROUP = 512            # moving free dim per matmul (one PSUM bank of fp32)
N_GROUPS = S // SQ_GROUP  # 4
N_CHUNKS = S // 128       # 16 key chunks

# Chunks whose exp runs on the DVE via the 16-bit Schraudolph bit-trick
# (i16 = arg*2^7/ln2 + B, bitcast bf16 ~= exp(arg), ~2% rms error) instead
# of the ACT engine -- balances the exp load across both engines.  Chosen
# away from c0/c1 (gp warmup) and c12..c15 (Z tail dependencies).
SCH_CHUNKS = frozenset(
    int(c) for c in os.environ.get("KERNEL_SCH_CHUNKS", "3,6,9").split(",") if c != ""
)
A16_SCH = 2.0 ** 7 / 0.6931471805599453        # 2^7 / ln2
B16_SCH = 127.0 * 2 ** 7 - 366393.0 / 65536.0  # bf16 exp bias - C_schraudolph

# matmul input dtype config
# "f32"  : plain fp32 (4 cycles/row, exact)
# "f32r" : fp32 replicated/round mode (1 cycle/row at N>=256, reduced precision)
QK_DTYPE = os.environ.get("KERNEL_QK_DTYPE", "bf16")
P_DTYPE = os.environ.get("KERNEL_P_DTYPE", "bf16")  # "f32" or "bf16"


def _qk_dt():
    if QK_DTYPE == "f32r":
        return mybir.dt.float32r
    if QK_DTYPE == "bf16":
        return BF16
    return FP32


def build_core_graph():
    """Build the per-core Bass graph (4 heads, full attention per head)."""
    nc = bacc.Bacc(
        "TRN2", target_bir_lowering=False, debug=False, enable_asserts=False
    )
    p_dt = BF16 if P_DTYPE == "bf16" else FP32

    q_d = nc.declare_dram_parameter("q", [HEADS_PER_CORE, S, D], FP32, isOutput=False)
    k_d = nc.declare_dram_parameter("k", [HEADS_PER_CORE, S, D], FP32, isOutput=False)
    v_d = nc.declare_dram_parameter("v", [HEADS_PER_CORE, S, D], FP32, isOutput=False)
    a_d = nc.declare_dram_parameter("alpha", [1, 1], FP32, isOutput=False)
    o_d = nc.declare_dram_parameter("out", [HEADS_PER_CORE, S, D], FP32, isOutput=True)

    q_ap, k_ap, v_ap, o_ap = q_d.ap(), k_d.ap(), v_d.ap(), o_d.ap()

    with tile.TileContext(nc) as tc, ExitStack() as ctx:
        const = ctx.enter_context(tc.tile_pool(name="const", bufs=1))
        nat = ctx.enter_context(tc.tile_pool(name="nat", bufs=4))
        qkt = ctx.enter_context(tc.tile_pool(name="qkt", bufs=2))
        vpool = ctx.enter_context(tc.tile_pool(name="vpool", bufs=2))
        ppool = ctx.enter_context(tc.tile_pool(name="ppool", bufs=8))
        epil = ctx.enter_context(tc.tile_pool(name="epil", bufs=3))
        outp = ctx.enter_context(tc.tile_pool(name="outp", bufs=4))
        # PSUM budget (8 banks): sp [128,1024] x2 bufs = 4, o [128,1024] x2
        # bufs = 4.  Z rides the just-freed sp tile of chunk 14 (WAR reuse
        # after its exp), so the O accumulator can double-buffer and the
        # epilogue never gates the next group's AV matmuls.
        ps_s = ctx.enter_context(tc.tile_pool(name="ps_s", bufs=2, space="PSUM"))
        ps_o = ctx.enter_context(tc.tile_pool(name="ps_o", bufs=2, space="PSUM"))
        dpool = ctx.enter_context(tc.tile_pool(name="dram", bufs=4, space="DRAM"))
        zqp = ctx.enter_context(tc.tile_pool(name="zqp", bufs=8))

        GP = 1024  # paired sq-group width (2 PSUM banks)
        N_GP = S // GP

        # PE "filler" tasks: one-bank transpose batches (Q/K input transposes
        # for the NEXT head, output transposes for the PREVIOUS sq-group)
        # injected between compute matmuls so the PE never idles long enough
        # to stall the pipeline or re-throttle HAM.
        fillers = []

        def _head_permuted(h):
            # heads loaded through the PE-transpose batch path use the
            # contiguous (permuted) layout; xbar-path heads are unpermuted
            return QK_DTYPE != "bf16" or h == 0

        def issue_head_loads(h):
            """Issue all DMA loads for head h and return (qT, kT, v_sb) plus
            filler tasks that transpose the loaded chunks into qT/kT."""
            qT = [qkt.tile([128, GP], _qk_dt(), tag=f"qT{i}", name=f"qT{i}") for i in range(2)]
            kT = [qkt.tile([128, GP], _qk_dt(), tag=f"kT{i}", name=f"kT{i}") for i in range(2)]
            v_sb = vpool.tile([128, N_CHUNKS, 128], p_dt, tag="v")
            perm = _head_permuted(h)

            def load_v():
                if perm:
                    v_src = v_ap[h].rearrange("(u p t) d -> p u t d", u=2, t=8)
                    v_dst = v_sb.rearrange("p (u t) d -> p u t d", u=2)
                else:
                    v_src = v_ap[h].rearrange("(c p) d -> p c d", p=128)
                    v_dst = v_sb
                if p_dt == FP32:
                    nc.sync.dma_start(out=v_dst, in_=v_src)
                else:
                    nc.gpsimd.dma_start(out=v_dst, in_=v_src)  # SWDGE casts
            tasks = []
            if QK_DTYPE == "bf16" and h > 0:
                # steady-state path: SWDGE cast f32->bf16 into DRAM scratch,
                # then whole-half 2-byte DMA xbar transposes — zero PE/DVE
                # work, fully hidden behind the previous head's compute
                qsc = dpool.tile([S, 128], BF16, tag="sc")
                ksc = dpool.tile([S, 128], BF16, tag="sc")
                nc.gpsimd.dma_start(out=qsc[:GP], in_=q_ap[h, :GP])
                nc.gpsimd.dma_start(out=ksc[:GP], in_=k_ap[h, :GP])
                nc.sync.dma_start_transpose(out=qT[0], in_=qsc[:GP])
                nc.sync.dma_start_transpose(out=kT[0], in_=ksc[:GP])
                nc.gpsimd.dma_start(out=ksc[GP:], in_=k_ap[h, GP:])
                nc.gpsimd.dma_start(out=qsc[GP:], in_=q_ap[h, GP:])
                nc.sync.dma_start_transpose(out=kT[1], in_=ksc[GP:])
                nc.sync.dma_start_transpose(out=qT[1], in_=qsc[GP:])
                load_v()
                return qT, kT, v_sb, tasks
            batches = [("q", 0), ("k", 0), ("k", 1), ("q", 1)]
            for name, hf in batches:
                src_ap, dsts = (q_ap, qT) if name == "q" else (k_ap, kT)
                xn = nat.tile([128, 8, 128], FP32, tag="qknat")
                eng = nc.sync if name == "q" else nc.scalar
                # contiguous 4KB-per-partition read; the implied permutation
                # (row p*8+t -> column t*128+p) is undone in the V layout and
                # the output store AP below
                eng.dma_start(
                    out=xn,
                    in_=src_ap[h, hf * GP:(hf + 1) * GP, :].rearrange(
                        "(p t) d -> p t d", p=128
                    ),
                )

                def mk(xn=xn, dsts=dsts, hf=hf):
                    tp = ps_s.tile([128, GP], FP32, tag="sp")
                    for j in range(8):
                        nc.tensor.transpose(
                            tp[:, j * 128:(j + 1) * 128], xn[:, j, :], ident
                        )
                    nc.vector.tensor_copy(out=dsts[hf], in_=tp)

                tasks.append(mk)
            load_v()
            return qT, kT, v_sb, tasks

        def emit_epilogue(h, gp, zp, op):
            """Normalize O^T/Z and write out.  bf16 path: normalize to bf16,
            transpose via the DMA xbar (SBUF->SBUF [128,512] -> [128,4,128]
            gives exactly the osb layout), then SWDGE cast-store — zero PE
            and near-zero DVE epilogue work beyond recip+mul."""
            rz = epil.tile([128, GP], FP32, tag="rz")
            bf = QK_DTYPE == "bf16"
            on_dt = BF16 if bf else FP32
            on = epil.tile([128, GP], on_dt, tag="on")
            for i in range(2):
                hs = slice(i * 512, (i + 1) * 512)
                nc.vector.reciprocal_approx_fast(out=rz[:, hs], in_=zp[:, hs])
                nc.vector.tensor_mul(on[:, hs], op[:, hs], rz[:, hs])

            def o_dst_ap(half4):
                if _head_permuted(h):
                    return o_ap[h, gp * GP:(gp + 1) * GP, :].rearrange(
                        "(p u t) d -> p u t d", p=128, u=2
                    )[:, half4, :, :]
                r0 = gp * GP + half4 * 512
                return o_ap[h, r0:r0 + 512, :].rearrange(
                    "(t p) d -> p t d", p=128
                )

            if bf:
                # xbar path: no PE work, so nothing goes on the filler list
                for half4 in range(2):
                    osb = outp.tile([128, 4, 128], BF16, tag="osb")
                    nc.sync.dma_start_transpose(
                        out=osb, in_=on[:, half4 * 512:(half4 + 1) * 512]
                    )
                    nc.gpsimd.dma_start(out=o_dst_ap(half4), in_=osb)
                return []

            tasks = []
            for half4 in range(2):

                def mk(h=h, gp=gp, on=on, half4=half4):
                    to = ps_s.tile([128, 512], FP32, tag="sp")
                    for t in range(4):
                        c0 = half4 * 512 + t * 128
                        nc.tensor.transpose(
                            to[:, t * 128:(t + 1) * 128], on[:, c0:c0 + 128], ident
                        )
                    osb = outp.tile([128, 4, 128], FP32, tag="osb")
                    nc.vector.tensor_copy(out=osb, in_=to)
                    nc.sync.dma_start(out=o_dst_ap(half4), in_=osb)

                tasks.append(mk)
            return tasks

        # ---- constants first: the tiny alpha DMA must not queue behind the
        # bulk head-0 loads (it gates bias -> exp table -> the whole pipeline)
        alpha_bc = const.tile([128, 1], FP32)
        nc.sync.dma_start(out=alpha_bc, in_=a_d.ap().to_broadcast([128, 1]))
        bias_sb = const.tile([128, 1], FP32)
        nc.vector.tensor_scalar_mul(bias_sb, alpha_bc, -BIAS_C)
        # Schraudolph affine constants with alpha folded in:
        # i16 = s*(alpha*A16) + (B16 - BIAS_C*alpha*A16)
        sch_scale = const.tile([128, 1], FP32)
        nc.vector.tensor_scalar_mul(sch_scale, alpha_bc, A16_SCH)
        sch_bias = const.tile([128, 1], FP32)
        nc.vector.tensor_scalar(
            sch_bias, alpha_bc, -BIAS_C * A16_SCH, B16_SCH,
            mybir.AluOpType.mult, mybir.AluOpType.add,
        )
        ident = const.tile([128, 128], FP32)
        make_identity(nc, ident)
        ident_bf = const.tile([128, 128], BF16)
        nc.vector.tensor_copy(out=ident_bf, in_=ident)
        ones_w = const.tile([128, 128], p_dt)
        nc.vector.memset(ones_w, 1.0)
        ones_bf = const.tile([128, 128], BF16)
        nc.vector.memset(ones_bf, 1.0)
        ones_r = const.tile([128, 128], mybir.dt.float32r)
        nc.vector.tensor_copy(out=ones_r, in_=ones_bf)

        # ---- head 0 loads
        qT, kT, v_sb, tasks = issue_head_loads(0)

        # PE warmup during the head-0 DMA wait: dummy transposes keep the
        # p-state ramp going so real work starts at full clock.  They only
        # depend on ident_bf, so they run as soon as the constants land.
        warm = ps_s.tile([128, 512], BF16, tag="sp")
        for _ in range(24):
            nc.tensor.transpose(warm[:, :128], ident_bf, ident_bf)

        for t in tasks[:2]:
            t()
        fillers.extend(tasks[2:])

        for h in range(HEADS_PER_CORE):
            nxt = None
            for gp in range(N_GP):
                if h + 1 < HEADS_PER_CORE and gp == N_GP - 1:
                    # prefetch next head: DMAs now, transposes become fillers
                    nxt = issue_head_loads(h + 1)
                    fillers.extend(nxt[3])
                op = ps_o.tile([128, GP], FP32, tag="o")
                zp_ref = [None]  # set to the chunk-14 sp tile (reused for Z)

                def consume(c, pt):
                    # AV accumulation for chunk c (after exp(c))
                    for i in range(2):
                        nc.tensor.matmul(
                            op[:, i * 512:(i + 1) * 512], lhsT=v_sb[:, c, :],
                            rhs=pt[:, i * 512:(i + 1) * 512],
                            start=(c == 0), stop=(c == N_CHUNKS - 1),
                        )

                def z_mm(zq, start, stop):
                    # Z accumulation: ones-matmul over a DVE-reduced tile,
                    # written into the dead chunk-14 S tile (freed by its exp)
                    zp = zp_ref[0]
                    for i in range(2):
                        nc.tensor.matmul(
                            zp[:, i * 512:(i + 1) * 512], lhsT=ones_bf,
                            rhs=zq[:, i * 512:(i + 1) * 512],
                            start=start, stop=stop,
                        )

                # software pipeline: PE issues S(c) while ACT exps c-1 and PE
                # consumes (AV) c-1.  Z denominator: DVE reduction tree over
                # exp tiles — chunks c0..c11 tree-reduce to one tile (3 ones-
                # matmul-equivalents saved vs pair mode), c12..c15 stay as two
                # pair tiles so the PE never waits long on a late DVE add.
                pend = []
                prev_pt = None
                l1 = []   # pair sums awaiting L2 combine (c0..c11 only)
                l2 = []   # quad sums awaiting L3 combine
                tree_a = [None]  # final c0..c11 reduction
                tail = []  # pair tiles fed directly to z_mm (c12/13, c14/15)
                for c in range(N_CHUNKS):
                    sp = ps_s.tile([128, GP], FP32, tag="sp")
                    kw = kT[c // 8][:, (c % 8) * 128:(c % 8) * 128 + 128]
                    for i in range(2):
                        nc.tensor.matmul(
                            sp[:, i * 512:(i + 1) * 512], lhsT=kw,
                            rhs=qT[gp][:, i * 512:(i + 1) * 512],
                            start=True, stop=True,
                        )
                    if c >= 2 and fillers:
                        fillers.pop(0)()
                    if c == 13:
                        # fold the leftover quad (c8..c11) into the tree
                        for rem in l2:
                            t2 = zqp.tile([128, GP], BF16, tag="zq")
                            nc.vector.tensor_add(t2, tree_a[0], rem)
                            tree_a[0] = t2
                        l2.clear()
                    if c == 14:
                        zp_ref[0] = sp
                    if len(pend) >= 2:
                        consume(*pend.pop(0))
                    pt = ppool.tile([128, GP], p_dt, tag="p")
                    if c in SCH_CHUNKS:
                        # exp on DVE: fused affine -> round-to-nearest i16,
                        # bitcast bf16 == Schraudolph exp of alpha*s - bias
                        nc.vector.tensor_scalar(
                            pt.bitcast(mybir.dt.int16), sp, sch_scale, sch_bias,
                            mybir.AluOpType.mult, mybir.AluOpType.add,
                        )
                    else:
                        nc.scalar.activation(
                            pt, sp, mybir.ActivationFunctionType.Exp,
                            bias=bias_sb, scale=alpha_bc,
                        )
                    if c % 2 == 1:
                        zq = zqp.tile([128, GP], BF16, tag="zq")
                        nc.vector.tensor_add(zq, prev_pt, pt)
                        if c < 12:
                            l1.append(zq)
                            if len(l1) == 2:
                                q = zqp.tile([128, GP], BF16, tag="zq")
                                nc.vector.tensor_add(q, l1[0], l1[1])
                                l1.clear()
                                l2.append(q)
                                if len(l2) == 2:
                                    t = zqp.tile([128, GP], BF16, tag="zq")
                                    nc.vector.tensor_add(t, l2[0], l2[1])
                                    l2.clear()
                                    if tree_a[0] is None:
                                        tree_a[0] = t
                                    else:
                                        t2 = zqp.tile([128, GP], BF16, tag="zq")
                                        nc.vector.tensor_add(t2, tree_a[0], t)
                                        tree_a[0] = t2
                        else:
                            tail.append(zq)
                    prev_pt = pt
                    pend.append((c, pt))
                # drain: AV(c14) first (it already waits exp(c14), covering
                # the Z tile's WAR on that same exp), then Z accumulation
                consume(*pend.pop(0))
                z_mm(tree_a[0], start=True, stop=False)
                z_mm(tail[0], start=False, stop=False)
                consume(*pend.pop(0))
                z_mm(tail[1], start=False, stop=True)
                fillers.extend(emit_epilogue(h, gp, zp_ref[0], op))
            if nxt is not None:
                qT, kT, v_sb = nxt[0], nxt[1], nxt[2]

        # drain remaining fillers (last head's output transposes)
        for t in fillers:
            t()

    nc.compile()
    return nc


_NC_CACHE = None
LAST_RESULT = {"exec_time_ns": None}


def _get_nc():
    global _NC_CACHE
    if _NC_CACHE is None:
        _NC_CACHE = build_core_graph()
    return _NC_CACHE


def kernel(q, k, v, alpha):
    q = np.ascontiguousarray(np.asarray(q, dtype=np.float32)).reshape(B * H, S, D)
    k = np.ascontiguousarray(np.asarray(k, dtype=np.float32)).reshape(B * H, S, D)
    v = np.ascontiguousarray(np.asarray(v, dtype=np.float32)).reshape(B * H, S, D)
    a = np.asarray(alpha, dtype=np.float32).reshape(1, 1)

    nc = _get_nc()
    in_maps = []
    for i in range(N_CORES):
        sl = slice(i * HEADS_PER_CORE, (i + 1) * HEADS_PER_CORE)
        in_maps.append({
            "q": np.ascontiguousarray(q[sl]),
            "k": np.ascontiguousarray(k[sl]),
            "v": np.ascontiguousarray(v[sl]),
            "alpha": a,
        })

    trace = os.environ.get("KERNEL_TRACE", "0") == "1"
    res = bass_utils.run_bass_kernel_spmd(
        nc, in_maps, core_ids=list(range(N_CORES)), trace=trace
    )
    LAST_RESULT["exec_time_ns"] = res.exec_time_ns
    LAST_RESULT["res"] = res

    out = np.stack([res.results[i]["out"] for i in range(N_CORES)])
    return out.reshape(B, H, S, D).astype(np.float32)

